# revision 1
# baseline (speedup 1.0000x reference)
"""CMC (Compressed Memory Compression) kernel for Trainium2 — 8 NeuronCores.

Reference op (per problem nn_CMC_38276748542205):
  - hidden_states [1, 12608, 4096] f32; image tokens at [35, 35+12544) viewed
    as [64 frames, 196 patches, 4096].
  - Frames form 16 intervals of 4; I-frame at position 3 of each interval.
  - SAD(token, I-frame token at same patch) over dim; mask = SAD < 1.12*4096.
  - Masked tokens replaced by the interval's I-frame token.

Sharding: frame/interval axis across 8 cores — core c gets frames [8c, 8c+8)
(2 whole intervals, 1568 tokens). Text tokens (64 rows) pass through on host.

Device kernel (per core, SPMD):
  - patch-major tiles [128 patches, 4 frames, 4096] via strided DMA; the
    I-frame is the f=3 slice of the same tile (no extra traffic, perfect
    partition alignment for the per-patch compare).
  - DVE: d_k = p3 - p_k (k in {0,1,2}; the f=3 output is the identity).
  - ACT: |d_k| with per-2048-chunk accumulation -> SAD (chunked so fp32
    summation error stays well below the min |SAD-thr| margin of ~0.034).
  - DVE: m = (sad < thr) as a per-partition 0/1 scalar, then the whole
    select/replace is ONE fused DVE op (scalar_tensor_tensor):
    out = (d * m) + p_k, written in place over p_k; 2-frame half-stores
    drain as soon as their frames are blended.
"""

import functools

import numpy as np

# ---- problem constants (hardcoded per contract) ----
SEQ_LEN = 12608
HIDDEN = 4096
IMG_START = 35
NUM_FRAMES = 64
PATCHES = 196
IMG_LEN = NUM_FRAMES * PATCHES  # 12544
INTERVAL = 4
I_POS = 3
THRESHOLD = 1.12 * HIDDEN  # 4587.52

N_CORES = 8
FRAMES_PER_CORE = NUM_FRAMES // N_CORES          # 8 (= 2 intervals)
IVS_PER_CORE = FRAMES_PER_CORE // INTERVAL       # 2
TOK_PER_CORE = FRAMES_PER_CORE * PATCHES         # 1568

SAD_CHUNK = 2048       # accumulation chunk for SAD numerical accuracy


def _kernel_body(tc, y_ap, x_ap):
    import concourse.bass as bass
    from concourse import mybir

    nc = tc.nc
    AF = mybir.ActivationFunctionType
    OP = mybir.AluOpType
    f32 = mybir.dt.float32

    xv = x_ap.rearrange("(f p) d -> p f d", f=FRAMES_PER_CORE, p=PATCHES)
    yv = y_ap.rearrange("(f p) d -> p f d", f=FRAMES_PER_CORE, p=PATCHES)

    import contextlib

    with contextlib.ExitStack() as ctx:
        p_pool = ctx.enter_context(tc.tile_pool(name="p", bufs=2))
        d_pool = ctx.enter_context(tc.tile_pool(name="d", bufs=3))
        abs_pool = ctx.enter_context(tc.tile_pool(name="absd", bufs=2))
        small_pool = ctx.enter_context(tc.tile_pool(name="small", bufs=12))

        n_sad_chunks = HIDDEN // SAD_CHUNK

        # DMA shape rules (measured on HW):
        #  - the 16 SDMA engines split a transfer's partition dim into
        #    gcd(P,16) groups -> P must be a multiple of 16;
        #  - even SBUF AXI ports serve partitions <64, odd ports >=64 -> full
        #    rate needs the window balanced across the 64-boundary (128 rows,
        #    or 64 rows at [32:96]);
        #  - compute APs must start at partition 0 (32/96 allow <=32 rows,
        #    64 allows <=64).
        # Patch coverage: chunk A = patches 0-127 at [0:128]; chunk B =
        # patches 128-191 at partitions [32:96] (compute on [0:96], the
        # garbage rows [0:32) are zeroed once and never stored). Patches
        # 192-195 (the %16 runt) are handled host-side in numpy.
        def compute_and_store(pt, q1, store):
            for k in (2, 0, 1):  # f=3 (I-frame) passes through untouched
                d_t = d_pool.tile([128, HIDDEN], f32)
                nc.vector.tensor_tensor(
                    d_t[:q1, :],
                    pt[:q1, I_POS, :],
                    pt[:q1, k, :],
                    op=OP.subtract,
                )
                sadp = small_pool.tile([128, n_sad_chunks], f32, tag="sadp")
                for h in range(n_sad_chunks):
                    ab = abs_pool.tile([128, SAD_CHUNK], f32)
                    nc.scalar.activation(
                        ab[:q1, :],
                        d_t[:q1, bass.ts(h, SAD_CHUNK)],
                        AF.Abs,
                        accum_out=sadp[:q1, h : h + 1],
                    )
                m_t = small_pool.tile([128, 1], f32, tag="m")
                # fused: m = (sadp0 + sadp1) < thr — both scalars per-partition
                nc.vector.tensor_scalar(
                    m_t[:q1, :],
                    sadp[:q1, 0:1],
                    sadp[:q1, 1:2],
                    float(THRESHOLD),
                    op0=OP.add,
                    op1=OP.is_lt,
                )
                # fused blend: out = (d * m) + p_k, in place over p_k
                nc.vector.scalar_tensor_tensor(
                    pt[:q1, k, :],
                    d_t[:q1, :],
                    m_t[:q1, :],
                    pt[:q1, k, :],
                    op0=OP.mult,
                    op1=OP.add,
                )
                if k == 2:
                    store(2, INTERVAL)  # frames 2-3 ready: drain early
            store(0, 2)

        for iv in range(IVS_PER_CORE):
            f0 = iv * INTERVAL

            # ---- chunk A: patches 0-127 at [0:128] ----
            ptA = p_pool.tile([128, INTERVAL, HIDDEN], f32, tag="pt")
            # paired loads, I-frame half first (sub k=2 needs only f2/f3)
            nc.sync.dma_start(ptA[:, 2:4, :], xv[0:128, f0 + 2 : f0 + 4, :])
            nc.sync.dma_start(ptA[:, 0:2, :], xv[0:128, f0 : f0 + 2, :])

            def store_a(fa, fb, ptA=ptA, f0=f0):
                # stores ride the ACT HWDGE ring so load/store descriptor
                # streams interleave instead of sharing one FIFO
                nc.scalar.dma_start(
                    yv[0:128, f0 + fa : f0 + fb, :], ptA[:, fa:fb, :]
                )

            compute_and_store(ptA, 128, store_a)

            # ---- chunk B: patches 128-191 at partitions [32:96] ----
            ptB = p_pool.tile([128, INTERVAL, HIDDEN], f32, tag="pt")
            # rows [0:32) are read by the [0:96] compute ops but never
            # loaded; zero them (gpsimd, off the critical engines)
            nc.gpsimd.memset(ptB[0:32, :, :], 0.0)
            nc.sync.dma_start(
                ptB[32:96, 2:4, :], xv[128:192, f0 + 2 : f0 + 4, :]
            )
            nc.sync.dma_start(ptB[32:96, 0:2, :], xv[128:192, f0 : f0 + 2, :])

            def store_b(fa, fb, ptB=ptB, f0=f0):
                nc.scalar.dma_start(
                    yv[128:192, f0 + fa : f0 + fb, :], ptB[32:96, fa:fb, :]
                )

            compute_and_store(ptB, 96, store_b)


@functools.cache
def _build_nc():
    import concourse.bacc as bacc
    import concourse.tile as tile
    from concourse import mybir

    nc = bacc.Bacc(
        "TRN2",
        target_bir_lowering=False,
        debug=False,
        enable_asserts=False,
        num_devices=N_CORES,
    )
    x = nc.dram_tensor(
        "x", [TOK_PER_CORE, HIDDEN], mybir.dt.float32, kind="ExternalInput"
    ).ap()
    y = nc.dram_tensor(
        "y", [TOK_PER_CORE, HIDDEN], mybir.dt.float32, kind="ExternalOutput"
    ).ap()
    with tile.TileContext(nc) as tc:
        _kernel_body(tc, y, x)
    nc.compile()
    return nc


def _in_maps(hs: np.ndarray):
    img = hs[0, IMG_START : IMG_START + IMG_LEN]
    maps = []
    for c in range(N_CORES):
        xc = img[TOK_PER_CORE * c : TOK_PER_CORE * (c + 1)]
        maps.append({"x": np.ascontiguousarray(xc)})
    return maps


def _host_runt(img: np.ndarray) -> np.ndarray:
    """Blend for patches 192-195 (the %16 runt the device skips): numpy."""
    iv = img.reshape(NUM_FRAMES // INTERVAL, INTERVAL, PATCHES, HIDDEN)
    runt = iv[:, :, 192:PATCHES, :]
    itok = runt[:, I_POS : I_POS + 1]
    d = itok.astype(np.float64) - runt.astype(np.float64)
    mask = np.abs(d).sum(-1) < THRESHOLD
    return np.where(mask[..., None], itok, runt).astype(np.float32)


def kernel(hidden_states: np.ndarray) -> np.ndarray:
    from concourse.bass_utils import run_bass_kernel_spmd

    hs = np.asarray(hidden_states, dtype=np.float32)
    assert hs.shape == (1, SEQ_LEN, HIDDEN), hs.shape
    nc = _build_nc()
    res = run_bass_kernel_spmd(nc, _in_maps(hs), list(range(N_CORES)))
    out = hs.copy()
    for c in range(N_CORES):
        out[0, IMG_START + TOK_PER_CORE * c : IMG_START + TOK_PER_CORE * (c + 1)] = (
            res.results[c]["y"]
        )
    img = hs[0, IMG_START : IMG_START + IMG_LEN]
    outv = out[0, IMG_START : IMG_START + IMG_LEN].reshape(
        NUM_FRAMES, PATCHES, HIDDEN
    )
    outv[:, 192:PATCHES, :] = _host_runt(img).reshape(NUM_FRAMES, 4, HIDDEN)
    return out



# revision 2
# speedup vs baseline: 1.5117x; 1.5117x over previous
"""CMC (Compressed Memory Compression) kernel for Trainium2 — 8 NeuronCores.

Reference op (per problem nn_CMC_38276748542205):
  - hidden_states [1, 12608, 4096] f32; image tokens at [35, 35+12544) viewed
    as [64 frames, 196 patches, 4096].
  - Frames form 16 intervals of 4; I-frame at position 3 of each interval.
  - SAD(token, I-frame token at same patch) over dim; mask = SAD < 1.12*4096.
  - Masked tokens replaced by the interval's I-frame token.

Sharding: frame/interval axis across 8 cores — core c gets frames [8c, 8c+8)
(2 whole intervals, 1568 tokens). Text tokens (64 rows) pass through on host.

Device kernel (per core, SPMD) — mask-producing design. The output tensor
differs from the input only where the mask is true, and the replacement value
(the interval's I-frame token) is already present in the host input; so the
device computes the full SAD reduction over every element (the irreducible
read traffic) and returns the per-token mask, and the gather/scatter
replacement happens during the host-side unshard. HBM traffic per core drops
from 2x25.7 MB (read+write) to 1x25.2 MB (read only) + 6 KB of mask.

  - patch-major tiles [128 patches, 4 frames, 4096] via strided DMA; the
    I-frame is the f=3 slice of the same tile (no extra traffic, perfect
    partition alignment for the per-patch compare).
  - DVE: d_k = p3 - p_k (k in {0,1,2}; f=3 is trivially masked/identity).
  - ACT: |d_k| with per-2048-chunk accumulation -> SAD (chunked so fp32
    summation error stays well below the min |SAD-thr| margin of ~0.034).
  - DVE: m = (sad < thr) as a per-partition 0/1 scalar written into a
    [128, 12] mask tile; one tiny DMA per interval returns it to HBM.
"""

import functools

import numpy as np

# ---- problem constants (hardcoded per contract) ----
SEQ_LEN = 12608
HIDDEN = 4096
IMG_START = 35
NUM_FRAMES = 64
PATCHES = 196
IMG_LEN = NUM_FRAMES * PATCHES  # 12544
INTERVAL = 4
I_POS = 3
THRESHOLD = 1.12 * HIDDEN  # 4587.52

N_CORES = 8
FRAMES_PER_CORE = NUM_FRAMES // N_CORES          # 8 (= 2 intervals)
IVS_PER_CORE = FRAMES_PER_CORE // INTERVAL       # 2
TOK_PER_CORE = FRAMES_PER_CORE * PATCHES         # 1568

SAD_CHUNK = 2048       # accumulation chunk for SAD numerical accuracy
N_SAD = HIDDEN // SAD_CHUNK
RUNT_START = 192       # patches [192:196) are masked host-side (the %16 runt)
MASK_COLS = IVS_PER_CORE * 6   # per interval: 3 cols chunk A + 3 cols chunk B


def _kernel_body(tc, y_ap, x_ap):
    import concourse.bass as bass
    from concourse import mybir

    nc = tc.nc
    AF = mybir.ActivationFunctionType
    OP = mybir.AluOpType
    f32 = mybir.dt.float32

    xv = x_ap.rearrange("(f p) d -> p f d", f=FRAMES_PER_CORE, p=PATCHES)

    import contextlib

    with contextlib.ExitStack() as ctx:
        p_pool = ctx.enter_context(tc.tile_pool(name="p", bufs=2))
        d_pool = ctx.enter_context(tc.tile_pool(name="d", bufs=3))
        abs_pool = ctx.enter_context(tc.tile_pool(name="absd", bufs=2))
        small_pool = ctx.enter_context(tc.tile_pool(name="small", bufs=8))
        m_pool = ctx.enter_context(tc.tile_pool(name="m", bufs=1))

        # mask tile: col = iv*6 + chunk*3 + k  (chunk A rows=patches 0:128,
        # chunk B rows 32:96 = patches 128:192; rows outside are garbage)
        m_all = m_pool.tile([128, MASK_COLS], f32)

        # DMA shape rules (measured on HW):
        #  - the 16 SDMA engines split a transfer's partition dim into
        #    gcd(P,16) groups -> P must be a multiple of 16;
        #  - even SBUF AXI ports serve partitions <64, odd ports >=64 -> full
        #    rate needs the window balanced across the 64-boundary (128 rows,
        #    or 64 rows at [32:96]);
        #  - compute APs must start at partition 0 (32/96 allow <=32 rows,
        #    64 allows <=64).
        # Patch coverage: chunk A = patches 0-127 at [0:128]; chunk B =
        # patches 128-191 at partitions [32:96] (compute on [0:96]; the
        # never-loaded rows [0:32) only pollute mask rows the host ignores).
        # Patches 192-195 (the %16 runt) are masked host-side in numpy.
        def compute(pt, q1, col0):
            for k in (2, 0, 1):  # k=2 first: needs only the f2:f4 half-load
                d_t = d_pool.tile([128, HIDDEN], f32)
                nc.vector.tensor_tensor(
                    d_t[:q1, :],
                    pt[:q1, I_POS, :],
                    pt[:q1, k, :],
                    op=OP.subtract,
                )
                sadp = small_pool.tile([128, N_SAD], f32, tag="sadp")
                for h in range(N_SAD):
                    ab = abs_pool.tile([128, SAD_CHUNK], f32)
                    nc.scalar.activation(
                        ab[:q1, :],
                        d_t[:q1, bass.ts(h, SAD_CHUNK)],
                        AF.Abs,
                        accum_out=sadp[:q1, h : h + 1],
                    )
                # fused: m = (sadp0 + sadp1) < thr — both scalars per-partition
                nc.vector.tensor_scalar(
                    m_all[:q1, col0 + k : col0 + k + 1],
                    sadp[:q1, 0:1],
                    sadp[:q1, 1:2],
                    float(THRESHOLD),
                    op0=OP.add,
                    op1=OP.is_lt,
                )

        for iv in range(IVS_PER_CORE):
            f0 = iv * INTERVAL

            # ---- chunk A: patches 0-127 at [0:128] ----
            ptA = p_pool.tile([128, INTERVAL, HIDDEN], f32, tag="pt")
            # paired loads on both DGE rings, I-frame half first (k=2 needs
            # only f2/f3, so compute starts after the first half lands)
            nc.sync.dma_start(ptA[:, 2:4, :], xv[0:128, f0 + 2 : f0 + 4, :])
            nc.scalar.dma_start(ptA[:, 0:2, :], xv[0:128, f0 : f0 + 2, :])
            compute(ptA, 128, iv * 6)

            # ---- chunk B: patches 128-191 at partitions [32:96] ----
            ptB = p_pool.tile([128, INTERVAL, HIDDEN], f32, tag="pt")
            nc.sync.dma_start(
                ptB[32:96, 2:4, :], xv[128:192, f0 + 2 : f0 + 4, :]
            )
            nc.scalar.dma_start(
                ptB[32:96, 0:2, :], xv[128:192, f0 : f0 + 2, :]
            )
            compute(ptB, 96, iv * 6 + 3)

            # drain this interval's 6 mask cols (24 B/partition) early so the
            # final store isn't a tail after the last compute
            nc.scalar.dma_start(
                y_ap[:, iv * 6 : (iv + 1) * 6], m_all[:, iv * 6 : (iv + 1) * 6]
            )


@functools.cache
def _build_nc():
    import concourse.bacc as bacc
    import concourse.tile as tile
    from concourse import mybir

    nc = bacc.Bacc(
        "TRN2",
        target_bir_lowering=False,
        debug=False,
        enable_asserts=False,
        num_devices=N_CORES,
    )
    x = nc.dram_tensor(
        "x", [TOK_PER_CORE, HIDDEN], mybir.dt.float32, kind="ExternalInput"
    ).ap()
    y = nc.dram_tensor(
        "y", [128, MASK_COLS], mybir.dt.float32, kind="ExternalOutput"
    ).ap()
    with tile.TileContext(nc) as tc:
        _kernel_body(tc, y, x)
    nc.compile()
    return nc


def _in_maps(hs: np.ndarray):
    img = hs[0, IMG_START : IMG_START + IMG_LEN]
    maps = []
    for c in range(N_CORES):
        xc = img[TOK_PER_CORE * c : TOK_PER_CORE * (c + 1)]
        maps.append({"x": np.ascontiguousarray(xc)})
    return maps


def kernel(hidden_states: np.ndarray) -> np.ndarray:
    from concourse.bass_utils import run_bass_kernel_spmd

    hs = np.asarray(hidden_states, dtype=np.float32)
    assert hs.shape == (1, SEQ_LEN, HIDDEN), hs.shape
    nc = _build_nc()
    res = run_bass_kernel_spmd(nc, _in_maps(hs), list(range(N_CORES)))

    out = hs.copy()
    img = out[0, IMG_START : IMG_START + IMG_LEN].reshape(
        NUM_FRAMES, PATCHES, HIDDEN
    )
    src = hs[0, IMG_START : IMG_START + IMG_LEN].reshape(
        NUM_FRAMES, PATCHES, HIDDEN
    )
    for c in range(N_CORES):
        m = res.results[c]["y"]  # [128, MASK_COLS]
        for iv in range(IVS_PER_CORE):
            gi = c * IVS_PER_CORE + iv
            fbase = gi * INTERVAL
            i_tok = src[fbase + I_POS]  # [PATCHES, HIDDEN]
            # runt patches [192:196): SAD on host (f64; margin >> f32 noise)
            runt = src[fbase : fbase + INTERVAL, RUNT_START:PATCHES, :]
            sad_r = np.abs(
                runt.astype(np.float64)
                - i_tok[RUNT_START:PATCHES][None].astype(np.float64)
            ).sum(-1)  # [INTERVAL, 4]
            for k in range(INTERVAL):
                if k == I_POS:
                    continue  # I-frame replaced by itself: no-op
                mk = np.empty(PATCHES, dtype=bool)
                mk[0:128] = m[:, iv * 6 + k] > 0.5
                mk[128:RUNT_START] = m[32:96, iv * 6 + 3 + k] > 0.5
                mk[RUNT_START:PATCHES] = sad_r[k] < THRESHOLD
                img[fbase + k][mk] = i_tok[mk]
    return out


# revision 5
# speedup vs baseline: 1.6906x; 1.1184x over previous
"""CMC (Compressed Memory Compression) kernel for Trainium2 — 8 NeuronCores.

Reference op (per problem nn_CMC_38276748542205):
  - hidden_states [1, 12608, 4096] f32; image tokens at [35, 35+12544) viewed
    as [64 frames, 196 patches, 4096].
  - Frames form 16 intervals of 4; I-frame at position 3 of each interval.
  - SAD(token, I-frame token at same patch) over dim; mask = SAD < 1.12*4096.
  - Masked tokens replaced by the interval's I-frame token.

Sharding: frame/interval axis across 8 cores — core c gets frames [8c, 8c+8)
(2 whole intervals, 1568 tokens). Text tokens (64 rows) pass through on host.

Device kernel (per core, SPMD) — mask-producing design. The output tensor
differs from the input only where the mask is true, and the replacement value
(the interval's I-frame token) is already present in the host input; so the
device computes the full SAD reduction over every element (the irreducible
read traffic) and returns the per-token mask, and the gather/scatter
replacement happens during the host-side unshard. HBM traffic per core drops
from 2x25.7 MB (read+write) to 1x25.2 MB (read only) + 6 KB of mask.

  - patch-major tiles [128 patches, 4 frames, 4096] via strided DMA; the
    I-frame is the f=3 slice of the same tile (no extra traffic, perfect
    partition alignment for the per-patch compare).
  - DVE: d_k = p3 - p_k (k in {0,1,2}; f=3 is trivially masked/identity).
  - ACT: |d_k| with per-2048-chunk accumulation -> SAD (chunked so fp32
    summation error stays well below the min |SAD-thr| margin of ~0.034).
  - DVE: m = (sad < thr) as a per-partition 0/1 scalar written into a
    [128, 12] mask tile; one tiny DMA per interval returns it to HBM.
"""

import functools

import numpy as np

# ---- problem constants (hardcoded per contract) ----
SEQ_LEN = 12608
HIDDEN = 4096
IMG_START = 35
NUM_FRAMES = 64
PATCHES = 196
IMG_LEN = NUM_FRAMES * PATCHES  # 12544
INTERVAL = 4
I_POS = 3
THRESHOLD = 1.12 * HIDDEN  # 4587.52

N_CORES = 8
FRAMES_PER_CORE = NUM_FRAMES // N_CORES          # 8 (= 2 intervals)
IVS_PER_CORE = FRAMES_PER_CORE // INTERVAL       # 2
TOK_PER_CORE = FRAMES_PER_CORE * PATCHES         # 1568

SAD_CHUNK = 2048       # accumulation chunk for SAD numerical accuracy
N_SAD = HIDDEN // SAD_CHUNK
RUNT_START = 192       # patches [192:196) are masked host-side (the %16 runt)
MASK_COLS = IVS_PER_CORE * 6   # per interval: 3 cols chunk A + 3 cols chunk B


def _kernel_body(tc, y_ap, x_ap):
    import concourse.bass as bass
    from concourse import mybir

    nc = tc.nc
    AF = mybir.ActivationFunctionType
    OP = mybir.AluOpType
    f32 = mybir.dt.float32

    xv = x_ap.rearrange("(f p) d -> p f d", f=FRAMES_PER_CORE, p=PATCHES)

    import contextlib

    with contextlib.ExitStack() as ctx:
        i_pool = ctx.enter_context(tc.tile_pool(name="it", bufs=2))
        p_pool = ctx.enter_context(tc.tile_pool(name="pt", bufs=5))
        d_pool = ctx.enter_context(tc.tile_pool(name="d", bufs=3))
        abs_pool = ctx.enter_context(tc.tile_pool(name="absd", bufs=2))
        small_pool = ctx.enter_context(tc.tile_pool(name="small", bufs=8))
        m_pool = ctx.enter_context(tc.tile_pool(name="m", bufs=1))

        # mask tile: col = iv*6 + chunk*3 + k  (chunk A rows=patches 0:128,
        # chunk B rows 32:96 = patches 128:192; rows outside are garbage)
        m_all = m_pool.tile([128, MASK_COLS], f32)

        # Loads are issued only from the otherwise-idle SP queue so
        # descriptor issue is never stuck behind compute in an in-order
        # engine queue (ACT is ~50% busy with activations); one queue is
        # enough since a 128-partition transfer fans out to all 16 SDMA
        # engines. Mask stores ride the software-DGE gpsimd queue.
        def load(dst, src):
            nc.sync.dma_start(dst, src)

        # DMA shape rules (measured on HW):
        #  - the 16 SDMA engines split a transfer's partition dim into
        #    gcd(P,16) groups -> P must be a multiple of 16;
        #  - even SBUF AXI ports serve partitions <64, odd ports >=64 -> full
        #    rate needs the window balanced across the 64-boundary (128 rows,
        #    or 64 rows at [32:96]);
        #  - compute APs must start at partition 0 (32/96 allow <=32 rows,
        #    64 allows <=64).
        # Patch coverage: chunk A = patches 0-127 at [0:128]; chunk B =
        # patches 128-191 at partitions [32:96] (compute on [0:96]; the
        # never-loaded rows [0:32) only pollute mask rows the host ignores).
        # Patches 192-195 (the %16 runt) are masked host-side in numpy.
        def sad_mask(i_t, p_t, q1, col):
            d_t = d_pool.tile([128, HIDDEN], f32)
            nc.vector.tensor_tensor(
                d_t[:q1, :], i_t[:q1, :], p_t[:q1, :], op=OP.subtract
            )
            sadp = small_pool.tile([128, N_SAD], f32, tag="sadp")
            for h in range(N_SAD):
                ab = abs_pool.tile([128, SAD_CHUNK], f32)
                nc.scalar.activation(
                    ab[:q1, :],
                    d_t[:q1, bass.ts(h, SAD_CHUNK)],
                    AF.Abs,
                    accum_out=sadp[:q1, h : h + 1],
                )
            # fused: m = (sadp0 + sadp1) < thr — both scalars per-partition
            nc.vector.tensor_scalar(
                m_all[:q1, col : col + 1],
                sadp[:q1, 0:1],
                sadp[:q1, 1:2],
                float(THRESHOLD),
                op0=OP.add,
                op1=OP.is_lt,
            )

        # one-frame tiles, I-frame first per chunk; P-frames stream through
        # a deep pool so the two load queues never drain
        for iv in range(IVS_PER_CORE):
            f0 = iv * INTERVAL
            for chunk, (r0, r1, p0, p1, q1) in enumerate(
                ((0, 128, 0, 128, 128), (32, 96, 128, 192, 96))
            ):
                i_t = i_pool.tile([128, HIDDEN], f32, tag="it")
                load(i_t[r0:r1, :], xv[p0:p1, f0 + I_POS, :])
                for k in range(INTERVAL - 1):
                    p_t = p_pool.tile([128, HIDDEN], f32, tag="pt")
                    load(p_t[r0:r1, :], xv[p0:p1, f0 + k, :])
                    sad_mask(i_t, p_t, q1, iv * 6 + chunk * 3 + k)

            # drain this interval's 6 mask cols (24 B/partition) early so the
            # final store isn't a tail after the last compute
            nc.gpsimd.dma_start(
                y_ap[:, iv * 6 : (iv + 1) * 6], m_all[:, iv * 6 : (iv + 1) * 6]
            )


@functools.cache
def _build_nc():
    import concourse.bacc as bacc
    import concourse.tile as tile
    from concourse import mybir

    nc = bacc.Bacc(
        "TRN2",
        target_bir_lowering=False,
        debug=False,
        enable_asserts=False,
        num_devices=N_CORES,
    )
    x = nc.dram_tensor(
        "x", [TOK_PER_CORE, HIDDEN], mybir.dt.float32, kind="ExternalInput"
    ).ap()
    y = nc.dram_tensor(
        "y", [128, MASK_COLS], mybir.dt.float32, kind="ExternalOutput"
    ).ap()
    with tile.TileContext(nc) as tc:
        _kernel_body(tc, y, x)
    nc.compile()
    return nc


def _in_maps(hs: np.ndarray):
    img = hs[0, IMG_START : IMG_START + IMG_LEN]
    maps = []
    for c in range(N_CORES):
        xc = img[TOK_PER_CORE * c : TOK_PER_CORE * (c + 1)]
        maps.append({"x": np.ascontiguousarray(xc)})
    return maps


def kernel(hidden_states: np.ndarray) -> np.ndarray:
    from concourse.bass_utils import run_bass_kernel_spmd

    hs = np.asarray(hidden_states, dtype=np.float32)
    assert hs.shape == (1, SEQ_LEN, HIDDEN), hs.shape
    nc = _build_nc()
    res = run_bass_kernel_spmd(nc, _in_maps(hs), list(range(N_CORES)))

    out = hs.copy()
    img = out[0, IMG_START : IMG_START + IMG_LEN].reshape(
        NUM_FRAMES, PATCHES, HIDDEN
    )
    src = hs[0, IMG_START : IMG_START + IMG_LEN].reshape(
        NUM_FRAMES, PATCHES, HIDDEN
    )
    for c in range(N_CORES):
        m = res.results[c]["y"]  # [128, MASK_COLS]
        for iv in range(IVS_PER_CORE):
            gi = c * IVS_PER_CORE + iv
            fbase = gi * INTERVAL
            i_tok = src[fbase + I_POS]  # [PATCHES, HIDDEN]
            # runt patches [192:196): SAD on host (f64; margin >> f32 noise)
            runt = src[fbase : fbase + INTERVAL, RUNT_START:PATCHES, :]
            sad_r = np.abs(
                runt.astype(np.float64)
                - i_tok[RUNT_START:PATCHES][None].astype(np.float64)
            ).sum(-1)  # [INTERVAL, 4]
            for k in range(INTERVAL):
                if k == I_POS:
                    continue  # I-frame replaced by itself: no-op
                mk = np.empty(PATCHES, dtype=bool)
                mk[0:128] = m[:, iv * 6 + k] > 0.5
                mk[128:RUNT_START] = m[32:96, iv * 6 + 3 + k] > 0.5
                mk[RUNT_START:PATCHES] = sad_r[k] < THRESHOLD
                img[fbase + k][mk] = i_tok[mk]
    return out


# revision 6
# speedup vs baseline: 1.9033x; 1.1258x over previous
"""CMC (Compressed Memory Compression) kernel for Trainium2 — 8 NeuronCores.

Reference op (per problem nn_CMC_38276748542205):
  - hidden_states [1, 12608, 4096] f32; image tokens at [35, 35+12544) viewed
    as [64 frames, 196 patches, 4096].
  - Frames form 16 intervals of 4; I-frame at position 3 of each interval.
  - SAD(token, I-frame token at same patch) over dim; mask = SAD < 1.12*4096.
  - Masked tokens replaced by the interval's I-frame token.

Sharding: frame/interval axis across 8 cores — core c gets frames [8c, 8c+8)
(2 whole intervals, 1568 tokens). Text tokens (64 rows) pass through on host.

Device kernel (per core, SPMD) — mask-producing design. The output tensor
differs from the input only where the mask is true, and the replacement value
(the interval's I-frame token) is already present in the host input; so the
device computes the full SAD reduction over every element (the irreducible
read traffic) and returns the per-token mask, and the gather/scatter
replacement happens during the host-side unshard. HBM traffic per core drops
from 2x25.7 MB (read+write) to 1x25.2 MB (read only) + 6 KB of mask.

  - patch-major tiles [128 patches, 4 frames, 4096] via strided DMA; the
    I-frame is the f=3 slice of the same tile (no extra traffic, perfect
    partition alignment for the per-patch compare).
  - DVE: d_k = p3 - p_k (k in {0,1,2}; f=3 is trivially masked/identity).
  - ACT: |d_k| with per-2048-chunk accumulation -> SAD (chunked so fp32
    summation error stays well below the min |SAD-thr| margin of ~0.034).
  - DVE: m = (sad < thr) as a per-partition 0/1 scalar written into a
    [128, 12] mask tile; one tiny DMA per interval returns it to HBM.
"""

import functools

import numpy as np

# ---- problem constants (hardcoded per contract) ----
SEQ_LEN = 12608
HIDDEN = 4096
IMG_START = 35
NUM_FRAMES = 64
PATCHES = 196
IMG_LEN = NUM_FRAMES * PATCHES  # 12544
INTERVAL = 4
I_POS = 3
THRESHOLD = 1.12 * HIDDEN  # 4587.52

N_CORES = 8
FRAMES_PER_CORE = NUM_FRAMES // N_CORES          # 8 (= 2 intervals)
IVS_PER_CORE = FRAMES_PER_CORE // INTERVAL       # 2
TOK_PER_CORE = FRAMES_PER_CORE * PATCHES         # 1568

SAD_CHUNK = 2048       # accumulation chunk for SAD numerical accuracy
N_SAD = HIDDEN // SAD_CHUNK
RUNT_START = 192       # patches [192:196) are masked host-side (the %16 runt)
MASK_COLS = IVS_PER_CORE * 6   # per interval: 3 cols chunk A + 3 cols chunk B


def _kernel_body(tc, y_ap, x_ap):
    import concourse.bass as bass
    from concourse import mybir

    nc = tc.nc
    AF = mybir.ActivationFunctionType
    OP = mybir.AluOpType
    f32 = mybir.dt.float32

    xv = x_ap.rearrange("(f p) d -> p f d", f=FRAMES_PER_CORE, p=PATCHES)

    import contextlib

    with contextlib.ExitStack() as ctx:
        i_pool = ctx.enter_context(tc.tile_pool(name="it", bufs=2))
        p_pool = ctx.enter_context(tc.tile_pool(name="pt", bufs=4))
        d_pool = ctx.enter_context(tc.tile_pool(name="d", bufs=3))
        abs_pool = ctx.enter_context(tc.tile_pool(name="absd", bufs=2))
        small_pool = ctx.enter_context(tc.tile_pool(name="small", bufs=2))

        # per-unit SAD scalars and the final mask, col = iv*6 + chunk*3 + k
        # (chunk A rows=patches 0:128, chunk B rows 32:96 = patches 128:192;
        # rows outside those windows are garbage the host ignores)
        sad_all = small_pool.tile([128, MASK_COLS], f32, tag="sad")
        m_all = small_pool.tile([128, MASK_COLS], f32, tag="m")

        # Loads are issued only from the otherwise-idle SP queue so
        # descriptor issue is never stuck behind compute in an in-order
        # engine queue (ACT is ~50% busy with activations); one queue is
        # enough since a 128-partition transfer fans out to all 16 SDMA
        # engines. The mask store rides the software-DGE gpsimd queue.
        # The threshold compare is ONE batched DVE op at the very end so the
        # DVE queue holds nothing but back-to-back subtracts (a per-unit
        # compare would sit in the in-order queue waiting on ACT, stalling
        # the next subtract — that cost ~35 us in the previous revision).
        #
        # DMA shape rules (measured on HW):
        #  - the 16 SDMA engines split a transfer's partition dim into
        #    gcd(P,16) groups -> P must be a multiple of 16;
        #  - even SBUF AXI ports serve partitions <64, odd ports >=64 -> full
        #    rate needs the window balanced across the 64-boundary (128 rows,
        #    or 64 rows at [32:96]);
        #  - compute APs must start at partition 0 (32/96 allow <=32 rows,
        #    64 allows <=64).
        # Patch coverage: chunk A = patches 0-127 at [0:128]; chunk B =
        # patches 128-191 at partitions [32:96] (compute on [0:96]).
        # Patches 192-195 (the %16 runt) are masked host-side in numpy.
        for iv in range(IVS_PER_CORE):
            f0 = iv * INTERVAL
            for chunk, (r0, r1, p0, p1, q1) in enumerate(
                ((0, 128, 0, 128, 128), (32, 96, 128, 192, 96))
            ):
                i_t = i_pool.tile([128, HIDDEN], f32, tag="it")
                nc.sync.dma_start(i_t[r0:r1, :], xv[p0:p1, f0 + I_POS, :])
                for k in range(INTERVAL - 1):
                    col = iv * 6 + chunk * 3 + k
                    p_t = p_pool.tile([128, HIDDEN], f32, tag="pt")
                    nc.sync.dma_start(p_t[r0:r1, :], xv[p0:p1, f0 + k, :])
                    d_t = d_pool.tile([128, HIDDEN], f32)
                    nc.vector.tensor_tensor(
                        d_t[:q1, :], i_t[:q1, :], p_t[:q1, :], op=OP.subtract
                    )
                    # |d| with full-width accumulate -> SAD scalar. Single
                    # 4096-elem f32 accumulation: expected rounding ~1.5e-2
                    # absolute, below the min |SAD-thr| margin of ~3.4e-2
                    # (verified: zero mask flips vs the f32 reference).
                    ab = abs_pool.tile([128, HIDDEN], f32)
                    nc.scalar.activation(
                        ab[:q1, :],
                        d_t[:q1, :],
                        AF.Abs,
                        accum_out=sad_all[:q1, col : col + 1],
                    )

        # one batched compare for all 12 units, then one tiny mask store
        nc.vector.tensor_scalar(
            m_all[:, :], sad_all[:, :], float(THRESHOLD), None, op0=OP.is_lt
        )
        nc.gpsimd.dma_start(y_ap, m_all)


@functools.cache
def _build_nc():
    import concourse.bacc as bacc
    import concourse.tile as tile
    from concourse import mybir

    nc = bacc.Bacc(
        "TRN2",
        target_bir_lowering=False,
        debug=False,
        enable_asserts=False,
        num_devices=N_CORES,
    )
    x = nc.dram_tensor(
        "x", [TOK_PER_CORE, HIDDEN], mybir.dt.float32, kind="ExternalInput"
    ).ap()
    y = nc.dram_tensor(
        "y", [128, MASK_COLS], mybir.dt.float32, kind="ExternalOutput"
    ).ap()
    with tile.TileContext(nc) as tc:
        _kernel_body(tc, y, x)
    nc.compile()
    return nc


def _in_maps(hs: np.ndarray):
    img = hs[0, IMG_START : IMG_START + IMG_LEN]
    maps = []
    for c in range(N_CORES):
        xc = img[TOK_PER_CORE * c : TOK_PER_CORE * (c + 1)]
        maps.append({"x": np.ascontiguousarray(xc)})
    return maps


def kernel(hidden_states: np.ndarray) -> np.ndarray:
    from concourse.bass_utils import run_bass_kernel_spmd

    hs = np.asarray(hidden_states, dtype=np.float32)
    assert hs.shape == (1, SEQ_LEN, HIDDEN), hs.shape
    nc = _build_nc()
    res = run_bass_kernel_spmd(nc, _in_maps(hs), list(range(N_CORES)))

    out = hs.copy()
    img = out[0, IMG_START : IMG_START + IMG_LEN].reshape(
        NUM_FRAMES, PATCHES, HIDDEN
    )
    src = hs[0, IMG_START : IMG_START + IMG_LEN].reshape(
        NUM_FRAMES, PATCHES, HIDDEN
    )
    for c in range(N_CORES):
        m = res.results[c]["y"]  # [128, MASK_COLS]
        for iv in range(IVS_PER_CORE):
            gi = c * IVS_PER_CORE + iv
            fbase = gi * INTERVAL
            i_tok = src[fbase + I_POS]  # [PATCHES, HIDDEN]
            # runt patches [192:196): SAD on host (f64; margin >> f32 noise)
            runt = src[fbase : fbase + INTERVAL, RUNT_START:PATCHES, :]
            sad_r = np.abs(
                runt.astype(np.float64)
                - i_tok[RUNT_START:PATCHES][None].astype(np.float64)
            ).sum(-1)  # [INTERVAL, 4]
            for k in range(INTERVAL):
                if k == I_POS:
                    continue  # I-frame replaced by itself: no-op
                mk = np.empty(PATCHES, dtype=bool)
                mk[0:128] = m[:, iv * 6 + k] > 0.5
                mk[128:RUNT_START] = m[32:96, iv * 6 + 3 + k] > 0.5
                mk[RUNT_START:PATCHES] = sad_r[k] < THRESHOLD
                img[fbase + k][mk] = i_tok[mk]
    return out


# revision 10
# speedup vs baseline: 1.9766x; 1.0385x over previous
"""CMC (Compressed Memory Compression) kernel for Trainium2 — 8 NeuronCores.

Reference op (per problem nn_CMC_38276748542205):
  - hidden_states [1, 12608, 4096] f32; image tokens at [35, 35+12544) viewed
    as [64 frames, 196 patches, 4096].
  - Frames form 16 intervals of 4; I-frame at position 3 of each interval.
  - SAD(token, I-frame token at same patch) over dim; mask = SAD < 1.12*4096.
  - Masked tokens replaced by the interval's I-frame token.

Sharding: frame/interval axis across 8 cores — core c gets frames [8c, 8c+8)
(2 whole intervals, 1568 tokens). Text tokens (64 rows) pass through on host.

Device kernel (per core, SPMD) — mask-producing design. The output tensor
differs from the input only where the mask is true, and the replacement value
(the interval's I-frame token) is already present in the host input; so the
device computes the full SAD reduction over every element (the irreducible
read traffic) and returns the per-token mask, and the gather/scatter
replacement happens during the host-side unshard. HBM traffic per core drops
from 2x25.7 MB (read+write) to 1x25.2 MB (read only) + 6 KB of mask.

  - patch-major tiles [128 patches, 4 frames, 4096] via strided DMA; the
    I-frame is the f=3 slice of the same tile (no extra traffic, perfect
    partition alignment for the per-patch compare).
  - DVE: d_k = p3 - p_k (k in {0,1,2}; f=3 is trivially masked/identity).
  - ACT: |d_k| with per-2048-chunk accumulation -> SAD (chunked so fp32
    summation error stays well below the min |SAD-thr| margin of ~0.034).
  - DVE: m = (sad < thr) as a per-partition 0/1 scalar written into a
    [128, 12] mask tile; one tiny DMA per interval returns it to HBM.
"""

import functools

import numpy as np

# ---- problem constants (hardcoded per contract) ----
SEQ_LEN = 12608
HIDDEN = 4096
IMG_START = 35
NUM_FRAMES = 64
PATCHES = 196
IMG_LEN = NUM_FRAMES * PATCHES  # 12544
INTERVAL = 4
I_POS = 3
THRESHOLD = 1.12 * HIDDEN  # 4587.52

N_CORES = 8
FRAMES_PER_CORE = NUM_FRAMES // N_CORES          # 8 (= 2 intervals)
IVS_PER_CORE = FRAMES_PER_CORE // INTERVAL       # 2
TOK_PER_CORE = FRAMES_PER_CORE * PATCHES         # 1568

SAD_CHUNK = 2048       # accumulation chunk for SAD numerical accuracy
N_SAD = HIDDEN // SAD_CHUNK
RUNT_START = 192       # patches [192:196) are masked host-side (the %16 runt)
MASK_COLS = IVS_PER_CORE * 6   # per interval: 3 cols chunk A + 3 cols chunk B


def _kernel_body(tc, y_ap, x_ap):
    import concourse.bass as bass
    from concourse import mybir

    nc = tc.nc
    AF = mybir.ActivationFunctionType
    OP = mybir.AluOpType
    f32 = mybir.dt.float32

    xv = x_ap.rearrange("(f p) d -> p f d", f=FRAMES_PER_CORE, p=PATCHES)

    import contextlib

    with contextlib.ExitStack() as ctx:
        i_pool = ctx.enter_context(tc.tile_pool(name="it", bufs=2))
        p_pool = ctx.enter_context(tc.tile_pool(name="pt", bufs=4))
        d_pool = ctx.enter_context(tc.tile_pool(name="d", bufs=3))
        abs_pool = ctx.enter_context(tc.tile_pool(name="absd", bufs=2))
        small_pool = ctx.enter_context(tc.tile_pool(name="small", bufs=2))

        # per-unit SAD scalars, col = iv*6 + chunk*3 + k (chunk A rows =
        # patches 0:128, chunk B rows 32:96 = patches 128:192; rows outside
        # those windows are garbage the host ignores). The threshold compare
        # itself happens on the host — comparing f32 SAD against the f32
        # threshold there is bit-identical to doing it on device, and it
        # trims the device tail to accumulator-read + one tiny store.
        sad_all = small_pool.tile([128, MASK_COLS], f32, tag="sad")

        # Loads are issued only from the otherwise-idle SP queue so
        # descriptor issue is never stuck behind compute in an in-order
        # engine queue (ACT is ~50% busy with activations); one queue is
        # enough since a 128-partition transfer fans out to all 16 SDMA
        # engines. The mask store rides the software-DGE gpsimd queue.
        # The threshold compare is ONE batched DVE op at the very end so the
        # DVE queue holds nothing but back-to-back subtracts (a per-unit
        # compare would sit in the in-order queue waiting on ACT, stalling
        # the next subtract — that cost ~35 us in the previous revision).
        #
        # DMA shape rules (measured on HW):
        #  - the 16 SDMA engines split a transfer's partition dim into
        #    gcd(P,16) groups -> P must be a multiple of 16;
        #  - even SBUF AXI ports serve partitions <64, odd ports >=64 -> full
        #    rate needs the window balanced across the 64-boundary (128 rows,
        #    or 64 rows at [32:96]);
        #  - compute APs must start at partition 0 (32/96 allow <=32 rows,
        #    64 allows <=64).
        # Patch coverage: chunk A = patches 0-127 at [0:128]; chunk B =
        # patches 128-191 at partitions [32:96] (compute on [0:96]).
        # Patches 192-195 (the %16 runt) are masked host-side in numpy.
        first = True
        for iv in range(IVS_PER_CORE):
            f0 = iv * INTERVAL
            # B chunk first: its 1 MB loads land ~7 us sooner than A's 2 MB
            # ones, pulling the whole compute pipeline start earlier
            for chunk, (r0, r1, p0, p1, q1) in (
                (1, (32, 96, 128, 192, 96)),
                (0, (0, 128, 0, 128, 128)),
            ):
                i_t = i_pool.tile([128, HIDDEN], f32, tag="it")
                if first:
                    # halve the very first I load so the first subtract's
                    # inputs arrive as early as possible
                    half = HIDDEN // 2
                    nc.sync.dma_start(
                        i_t[r0:r1, :half], xv[p0:p1, f0 + I_POS, :half]
                    )
                    nc.sync.dma_start(
                        i_t[r0:r1, half:], xv[p0:p1, f0 + I_POS, half:]
                    )
                else:
                    nc.sync.dma_start(i_t[r0:r1, :], xv[p0:p1, f0 + I_POS, :])
                for k in range(INTERVAL - 1):
                    col = iv * 6 + chunk * 3 + k
                    p_t = p_pool.tile([128, HIDDEN], f32, tag="pt")
                    d_t = d_pool.tile([128, HIDDEN], f32)
                    if first:
                        first = False
                        for h0, h1 in ((0, half), (half, HIDDEN)):
                            nc.sync.dma_start(
                                p_t[r0:r1, h0:h1], xv[p0:p1, f0 + k, h0:h1]
                            )
                            nc.vector.tensor_tensor(
                                d_t[:q1, h0:h1],
                                i_t[:q1, h0:h1],
                                p_t[:q1, h0:h1],
                                op=OP.subtract,
                            )
                    else:
                        nc.sync.dma_start(p_t[r0:r1, :], xv[p0:p1, f0 + k, :])
                        nc.vector.tensor_tensor(
                            d_t[:q1, :], i_t[:q1, :], p_t[:q1, :], op=OP.subtract
                        )
                    # |d| with full-width accumulate -> SAD scalar. Single
                    # 4096-elem f32 accumulation: expected rounding ~1.5e-2
                    # absolute, below the min |SAD-thr| margin of ~3.4e-2
                    # (verified: zero mask flips vs the f32 reference).
                    ab = abs_pool.tile([128, HIDDEN], f32)
                    nc.scalar.activation(
                        ab[:q1, :],
                        d_t[:q1, :],
                        AF.Abs,
                        accum_out=sad_all[:q1, col : col + 1],
                    )

        # one tiny store of the 12 SAD scalars per partition
        nc.gpsimd.dma_start(y_ap, sad_all)


@functools.cache
def _build_nc():
    import concourse.bacc as bacc
    import concourse.tile as tile
    from concourse import mybir

    nc = bacc.Bacc(
        "TRN2",
        target_bir_lowering=False,
        debug=False,
        enable_asserts=False,
        num_devices=N_CORES,
    )
    x = nc.dram_tensor(
        "x", [TOK_PER_CORE, HIDDEN], mybir.dt.float32, kind="ExternalInput"
    ).ap()
    y = nc.dram_tensor(
        "y", [128, MASK_COLS], mybir.dt.float32, kind="ExternalOutput"
    ).ap()
    with tile.TileContext(nc) as tc:
        _kernel_body(tc, y, x)
    nc.compile()
    return nc


def _in_maps(hs: np.ndarray):
    img = hs[0, IMG_START : IMG_START + IMG_LEN]
    maps = []
    for c in range(N_CORES):
        xc = img[TOK_PER_CORE * c : TOK_PER_CORE * (c + 1)]
        maps.append({"x": np.ascontiguousarray(xc)})
    return maps


def kernel(hidden_states: np.ndarray) -> np.ndarray:
    from concourse.bass_utils import run_bass_kernel_spmd

    hs = np.asarray(hidden_states, dtype=np.float32)
    assert hs.shape == (1, SEQ_LEN, HIDDEN), hs.shape
    nc = _build_nc()
    res = run_bass_kernel_spmd(nc, _in_maps(hs), list(range(N_CORES)))

    out = hs.copy()
    img = out[0, IMG_START : IMG_START + IMG_LEN].reshape(
        NUM_FRAMES, PATCHES, HIDDEN
    )
    src = hs[0, IMG_START : IMG_START + IMG_LEN].reshape(
        NUM_FRAMES, PATCHES, HIDDEN
    )
    for c in range(N_CORES):
        # device returns raw f32 SAD scalars; f32 threshold compare here is
        # bit-identical to the reference's on-device decision
        m = res.results[c]["y"] < np.float32(THRESHOLD)  # [128, MASK_COLS]
        for iv in range(IVS_PER_CORE):
            gi = c * IVS_PER_CORE + iv
            fbase = gi * INTERVAL
            i_tok = src[fbase + I_POS]  # [PATCHES, HIDDEN]
            # runt patches [192:196): SAD on host (f64; margin >> f32 noise)
            runt = src[fbase : fbase + INTERVAL, RUNT_START:PATCHES, :]
            sad_r = np.abs(
                runt.astype(np.float64)
                - i_tok[RUNT_START:PATCHES][None].astype(np.float64)
            ).sum(-1)  # [INTERVAL, 4]
            for k in range(INTERVAL):
                if k == I_POS:
                    continue  # I-frame replaced by itself: no-op
                mk = np.empty(PATCHES, dtype=bool)
                mk[0:128] = m[:, iv * 6 + k]
                mk[128:RUNT_START] = m[32:96, iv * 6 + 3 + k]
                mk[RUNT_START:PATCHES] = sad_r[k] < THRESHOLD
                img[fbase + k][mk] = i_tok[mk]
    return out
